# revision 4
# baseline (speedup 1.0000x reference)
"""ARD-RBF kernel matrix on 8 TRN2 NeuronCores.

Math (reference):
    alpha = softmax(alpha_raw^2)            (D,)
    var   = variance_raw^2                  scalar
    sq_ij = sum_d alpha_d (x1_id - x2_jd)^2
    out   = var * exp(-0.5 * sq)            (N, M) f32

Device formulation (rows of x1 sharded 8 ways; per core):
    out_ij = exp( (16*cross_ij)/16 - 0.5*ra_i + ln var ) * exp(-0.5*rb_j)
    cross  = x1 @ (alpha * x2)^T            fp16 matmul, f32 PSUM accum
b = 16*alpha*x2 is pre-scaled (fp16 subnormal avoidance); ACT's free
input scale (1/16) undoes it.

The pipeline is paced by ScalarE (ACT), the only engine that can do exp:
32 x [128,2048] Exp ops at (2048+352)/1.2 = ~2.0us each = ~64us/core.
Everything else must hide under it:
  PE      8 fp16 N=512 matmuls/group; LDWEIGHTS deduped (ldweights=False
          on same-weight matmuls) so MMs stream back-to-back at ~216ns.
  DVE     fp16 column-scale by exp(-0.5 rb): ~1.25us/group (2x mode).
  DMA     16MB out + 6.5MB in; inputs split across THREE rings (sync-HW,
          act-HW, gpsimd-SW) so the ~1.5MB the first two groups need
          lands as early as the rings allow.

ot/erb are fp16 (not bf16): 3x lower rounding error (rel err ~2.7e-3).

Startup: engine boot + ring-init is ~5-6us; junk matmuls keep PE busy
(HAM p-state) until data arrives; G0 is processed in 4 512-col chunks so
ACT's first op issues as soon as the first 320KB of input lands.
Tail: G30/G31 mul+DMA are split in halves; G31's ACT is split in two
1024-col ops so the drain pipeline overlaps.
"""

import math
import sys

import numpy as np

import ml_dtypes  # noqa: F401

if "/opt/trn_rl_repo" not in sys.path:
    sys.path.insert(0, "/opt/trn_rl_repo")

N, M, D = 8192, 8192, 256
NCORES = 8
NS = N // NCORES          # 1024 rows of x1 per core
P = 128                   # partitions
KT = D // P               # 2 k-tiles
NG = 4                    # x2 column groups
JG = M // NG              # 2048 cols per group
NJ = 512                  # matmul moving free dim (1 PSUM bank)
NT = NS // P              # 8 row tiles per core
NGRP = NG * NT            # 32 groups

SCALE_B = 16.0            # b pre-scale; ACT applies 1/SCALE_B

_F16 = np.float16

_compiled = None

WARM = 9                  # junk warmup matmuls (N=256) to keep PE busy pre-data


def _build():
    import concourse.bass as bass
    import concourse.mybir as mybir
    from concourse.env import get_walrus_max_sem_num
    from contextlib import ExitStack

    # Shrink the kernel semaphore pool: the module epilogue emits per-sem
    # reset ops over the whole pool; this kernel uses <30, so a 40-wide
    # pool cuts the epilogue cost.
    base = get_walrus_max_sem_num()
    bass.get_kernel_semaphore_range = lambda: range(base, base + 40)

    dt = mybir.dt
    nc = bass.Bass()

    x1d = nc.declare_dram_parameter("x1d", [P, KT * NS], dt.float16, isOutput=False)
    x2d = nc.declare_dram_parameter("x2d", [P, NG * KT * JG], dt.float16, isOutput=False)
    rbd = nc.declare_dram_parameter("rbd", [P, M], dt.float16, isOutput=False)
    biad = nc.declare_dram_parameter("biad", [P, NT], dt.float32, isOutput=False)
    outd = nc.declare_dram_parameter("out", [NS, M], dt.float16, isOutput=True)

    exp_f = mybir.ActivationFunctionType.Exp
    njc = JG // NJ            # 4 matmul column chunks per group
    OTN = 8
    INV_SB = 1.0 / SCALE_B

    # per-group output units (mul + dma granularity); tail groups split
    def units_of(G):
        if G >= NGRP - 2:
            h = JG // 2
            return [(0, h), (h, JG)]
        return [(0, JG)]

    # ACT op units per group
    def act_units(G):
        if G == 0:
            return [(c * NJ, (c + 1) * NJ) for c in range(njc)]
        if G == NGRP - 1:
            h = JG // 2
            return [(0, h), (h, JG)]
        return [(0, JG)]

    # cumulative acs (ACT-unit) count after group G fully processed
    acs_after = {}
    c = 0
    for G in range(NGRP):
        c += len(act_units(G))
        acs_after[G] = c
    # cumulative vcs (mul-unit) count
    vcs_after = {}
    c = 0
    for G in range(NGRP):
        c += len(units_of(G))
        vcs_after[G] = c
    # per-slot cumulative DMA chunk counts (for ACT slot-reuse waits)
    slot_cum = {}  # (slot, through_group) -> chunks
    for s in range(OTN):
        tot = 0
        for G in range(s, NGRP, OTN):
            tot += len(units_of(G))
            slot_cum[(s, G)] = tot
    slot_total = {s: slot_cum[(s, max(g for g in range(NGRP) if g % OTN == s))]
                  for s in range(OTN)}

    def gt(G):
        return divmod(G, NT)

    pes_after = lambda G: (njc if G == 0 else njc + G)  # G0 incs per chunk

    with ExitStack() as _ctx:
        ec = _ctx.enter_context
        x1s = ec(nc.sbuf_tensor("x1s", [P, KT * NS], dt.float16))
        x2s = ec(nc.sbuf_tensor("x2s", [P, NG * KT * JG], dt.float16))
        bis = ec(nc.sbuf_tensor("bis", [P, NT], dt.float32))
        erb = ec(nc.sbuf_tensor("erb", [P, M], dt.float16))
        ots = [ec(nc.sbuf_tensor(f"ot{i}", [P, JG], dt.float16)) for i in range(OTN)]
        wrm = ec(nc.sbuf_tensor("wrm", [P, P + 256], dt.float16))  # uninit junk
        scr = ec(nc.sbuf_tensor("scr", [1, 32], dt.float32))       # table preload
        ps0 = ec(nc.psum_tensor("ps0", [P, JG], dt.float32))
        ps1 = ec(nc.psum_tensor("ps1", [P, JG], dt.float32))
        pss = [ps0, ps1]

        s_bia = ec(nc.semaphore("s_bia"))    # bia               (full 16)
        s_x1a = ec(nc.semaphore("s_x1a"))    # x1 k0 t0          (16)
        s_x1b = ec(nc.semaphore("s_x1b"))    # x1 k1 t0          (16)
        s_x1r = ec(nc.semaphore("s_x1r"))    # x1 rest k0+k1     (32)
        s_x2a = ec(nc.semaphore("s_x2a"))    # x2 g0 k0 j0       (16)
        s_x2b = ec(nc.semaphore("s_x2b"))    # x2 g0 k1 j0       (16)
        s_x2c1 = ec(nc.semaphore("s_x2c1"))  # x2 g0 k0 j1       (16)
        s_x2c2 = ec(nc.semaphore("s_x2c2"))  # x2 g0 k1 j1       (16)
        s_x2d = ec(nc.semaphore("s_x2d"))    # x2 g0 k0 j2-3     (16)
        s_x2e = ec(nc.semaphore("s_x2e"))    # x2 g0 k1 j2-3     (16)
        s_x2g1 = ec(nc.semaphore("s_x2g1"))  # x2 g1             (16)
        s_x2g23 = ec(nc.semaphore("s_x2g23"))  # x2 g2 (+16) g3 (+16)
        ebA = ec(nc.semaphore("ebA"))        # erb g0            (16)
        ebB = ec(nc.semaphore("ebB"))        # erb g1            (16)
        ebC = ec(nc.semaphore("ebC"))        # erb g2 (+16) g3 (+16)... one DMA (16)
        pes = ec(nc.semaphore("pes"))
        acs = ec(nc.semaphore("acs"))
        vcs = ec(nc.semaphore("vcs"))
        dps = [ec(nc.semaphore(f"dp{i}")) for i in range(OTN)]
        block = ec(nc.Block())

        ebs_l = [ebA, ebB, ebC, ebC]
        ebs_n = [16, 16, 16, 32]

        @block.sync
        def _(sync):
            # critical-path inputs on the sync HW ring, then output chunks
            sync.dma_start(bis[:, :], biad[:, :]).then_inc(s_bia, 16)
            sync.dma_start(x1s[:, 0:P], x1d[:, 0:P]).then_inc(s_x1a, 16)
            sync.dma_start(x1s[:, NS:NS + P], x1d[:, NS:NS + P]).then_inc(s_x1b, 16)
            sync.dma_start(x2s[:, 0:NJ], x2d[:, 0:NJ]).then_inc(s_x2a, 16)
            sync.dma_start(x2s[:, JG:JG + NJ], x2d[:, JG:JG + NJ]).then_inc(s_x2b, 16)
            sync.dma_start(x1s[:, P:NS], x1d[:, P:NS]).then_inc(s_x1r, 16)
            sync.dma_start(x1s[:, NS + P:2 * NS], x1d[:, NS + P:2 * NS]).then_inc(s_x1r, 16)
            for G in range(NGRP):
                g, t = gt(G)
                us = units_of(G)
                for n, (lo, hi) in enumerate(us):
                    sync.wait_ge(vcs, vcs_after[G] - len(us) + 1 + n)
                    sync.dma_start(
                        outd[t * P:(t + 1) * P, g * JG + lo:g * JG + hi],
                        ots[G % OTN][:, lo:hi],
                    ).then_inc(dps[G % OTN], 16)
            for s in range(OTN):
                sync.wait_ge(dps[s], 16 * slot_total[s])

        @block.gpsimd
        def _(gpsimd):
            # second parallel ring (SW DGE): g0 j1 pieces, erb, then bulk x2
            gpsimd.dma_start(x2s[:, NJ:2 * NJ], x2d[:, NJ:2 * NJ]).then_inc(s_x2c1, 16)
            gpsimd.dma_start(x2s[:, JG + NJ:JG + 2 * NJ],
                             x2d[:, JG + NJ:JG + 2 * NJ]).then_inc(s_x2c2, 16)
            gpsimd.dma_start(erb[:, 0:JG], rbd[:, 0:JG]).then_inc(ebA, 16)
            gpsimd.dma_start(x2s[:, 2 * JG:4 * JG], x2d[:, 2 * JG:4 * JG]).then_inc(s_x2g1, 16)
            gpsimd.dma_start(erb[:, JG:2 * JG], rbd[:, JG:2 * JG]).then_inc(ebB, 16)
            gpsimd.dma_start(x2s[:, 4 * JG:6 * JG], x2d[:, 4 * JG:6 * JG]).then_inc(s_x2g23, 16)
            gpsimd.dma_start(x2s[:, 6 * JG:8 * JG], x2d[:, 6 * JG:8 * JG]).then_inc(s_x2g23, 16)
            gpsimd.dma_start(erb[:, 2 * JG:4 * JG], rbd[:, 2 * JG:4 * JG]).then_inc(ebC, 32)

        @block.tensor
        def _(tensor):
            # junk matmuls keep the PE busy (HAM activity window) while
            # inputs stream in; they write ps1 which G1 overwrites later.
            for _ in range(WARM):
                tensor.matmul(ps1[:, 0:256], wrm[:, 0:P], wrm[:, P:P + 256],
                              start=True, stop=True)
            # G0: j-outer, (k0,k1) per 512-col chunk -> ACT can drain per chunk
            for j in range(njc):
                for k in range(KT):
                    if j == 0 and k == 0:
                        tensor.wait_ge(s_x1a, 16)
                        tensor.wait_ge(s_x2a, 16)
                    if j == 0 and k == 1:
                        tensor.wait_ge(s_x1b, 16)
                        tensor.wait_ge(s_x2b, 16)
                    if j == 1 and k == 0:
                        tensor.wait_ge(s_x2c1, 16)
                    if j == 1 and k == 1:
                        tensor.wait_ge(s_x2c2, 16)
                    if j == 2 and k == 0:
                        tensor.wait_ge(s_x2d, 16)
                    if j == 2 and k == 1:
                        tensor.wait_ge(s_x2e, 16)
                    mm = tensor.matmul(
                        ps0[:, j * NJ:(j + 1) * NJ],
                        x1s[:, k * NS: k * NS + P],
                        x2s[:, k * JG + j * NJ: k * JG + (j + 1) * NJ],
                        start=(k == 0),
                        stop=(k == KT - 1),
                    )
                mm.then_inc(pes)
            for G in range(1, NGRP):
                g, t = gt(G)
                if G == 1:
                    tensor.wait_ge(s_x1r, 32)
                if G == NT:
                    tensor.wait_ge(s_x2g1, 16)
                if G == 2 * NT:
                    tensor.wait_ge(s_x2g23, 16)
                if G == 3 * NT:
                    tensor.wait_ge(s_x2g23, 32)
                if G >= 2:
                    tensor.wait_ge(acs, acs_after[G - 2])  # psum half free
                ps = pss[G % 2]
                for k in range(KT):
                    for j in range(njc):
                        mm = tensor.matmul(
                            ps[:, j * NJ:(j + 1) * NJ],
                            x1s[:, k * NS + t * P: k * NS + (t + 1) * P],
                            x2s[:, (g * KT + k) * JG + j * NJ:
                                   (g * KT + k) * JG + (j + 1) * NJ],
                            start=(k == 0),
                            stop=(k == KT - 1),
                        )
                        if j > 0:
                            # same stationary weights as previous matmul:
                            # suppress the redundant LDWEIGHTS
                            inst = mm.ins
                            inst = inst[0] if isinstance(inst, (list, tuple)) else inst
                            inst.ldweights = False
                mm.then_inc(pes)

        @block.scalar
        def _(scalar):
            # touch Exp early (reads junk SBUF, no DMA dependency) so the
            # ACT_TABLE_LOAD overlaps engine boot + input DMAs
            scalar.activation(scr[0:1, 16:32], scr[0:1, 0:16], exp_f)
            # third parallel ring (ACT HW DGE): g0 j2-3 pieces
            scalar.dma_start(x2s[:, 2 * NJ:JG], x2d[:, 2 * NJ:JG]).then_inc(s_x2d, 16)
            scalar.dma_start(x2s[:, JG + 2 * NJ:2 * JG],
                             x2d[:, JG + 2 * NJ:2 * JG]).then_inc(s_x2e, 16)
            scalar.wait_ge(s_bia, 16)
            for G in range(NGRP):
                g, t = gt(G)
                if G >= OTN and G % 4 == 0:
                    # batched slot-reuse wait covering the next 4 groups
                    for i in range(4):
                        Gp = G + i
                        sl = Gp % OTN
                        scalar.wait_ge(dps[sl], 16 * slot_cum[(sl, Gp - OTN)])
                for n, (lo, hi) in enumerate(act_units(G)):
                    if G == 0:
                        scalar.wait_ge(pes, n + 1)
                    elif n == 0:
                        scalar.wait_ge(pes, pes_after(G))
                    scalar.activation(
                        ots[G % OTN][:, lo:hi],
                        pss[G % 2][:, lo:hi],
                        exp_f,
                        bias=bis[:, t:t + 1],
                        scale=INV_SB,
                    ).then_inc(acs)

        @block.vector
        def _(vector):
            for G in range(NGRP):
                g, t = gt(G)
                if t == 0:
                    vector.wait_ge(ebs_l[g], ebs_n[g])
                us = units_of(G)
                aus = act_units(G)
                for n, (lo, hi) in enumerate(us):
                    # wait until ACT has produced at least through `hi`
                    need = acs_after[G] - len(aus)
                    covered = 0
                    for (alo, ahi) in aus:
                        need += 1
                        covered = ahi
                        if covered >= hi:
                            break
                    vector.wait_ge(acs, need)
                    vector.tensor_mul(ots[G % OTN][:, lo:hi], ots[G % OTN][:, lo:hi],
                                      erb[:, g * JG + lo:g * JG + hi]).then_inc(vcs)

    return nc


def _prep(x1, x2, alpha_raw, variance_raw):
    x1 = np.ascontiguousarray(np.asarray(x1, dtype=np.float32))
    x2 = np.ascontiguousarray(np.asarray(x2, dtype=np.float32))
    ar = np.asarray(alpha_raw, dtype=np.float64).reshape(-1)
    vr = np.asarray(variance_raw, dtype=np.float64).reshape(-1)

    a2 = ar * ar
    e = np.exp(a2 - a2.max())
    alpha = e / e.sum()                                   # (D,) f64
    var = float(vr[0]) ** 2
    if var > 0.0:
        logvar, post = math.log(var), None
    else:
        logvar, post = 0.0, var

    b = (SCALE_B * alpha[None, :]) * x2.astype(np.float64)  # (M, D)
    x2tm = b.T.reshape(KT, P, M).astype(_F16)             # [k, p, col]
    # device layout: col index = g*(KT*JG) + k*JG + j
    x2c = np.ascontiguousarray(
        x2tm.reshape(KT, P, NG, JG).transpose(1, 2, 0, 3).reshape(P, NG * KT * JG))
    x1tm = x1.T.reshape(KT, P, N).astype(_F16)            # [k, p, row]

    ra = (x1.astype(np.float64) ** 2) @ alpha             # (N,)
    rb = (x2.astype(np.float64) ** 2) @ alpha             # (M,)
    bia = (-0.5 * ra + logvar).astype(np.float32)         # (N,)
    rbrow = np.exp(-0.5 * rb).astype(_F16).reshape(1, M)
    rbd = np.ascontiguousarray(np.broadcast_to(rbrow, (P, M)))

    in_maps = []
    for c in range(NCORES):
        sl = slice(c * NS, (c + 1) * NS)
        x1c = np.ascontiguousarray(
            np.concatenate([x1tm[0][:, sl], x1tm[1][:, sl]], axis=1))
        bia2 = np.ascontiguousarray(
            bia[sl].reshape(NT, P).T.astype(np.float32))   # [p, t]
        in_maps.append({
            "x1d": x1c,
            "x2d": x2c,
            "rbd": rbd,
            "biad": bia2,
        })
    return in_maps, post


def _run(in_maps, trace=False):
    global _compiled
    from concourse.bass_utils import run_bass_kernel_spmd

    if _compiled is None:
        _compiled = _build()
    return run_bass_kernel_spmd(
        _compiled, in_maps, core_ids=list(range(NCORES)), trace=trace
    )


def kernel(x1, x2, alpha_raw, variance_raw):
    in_maps, post = _prep(x1, x2, alpha_raw, variance_raw)
    res = _run(in_maps)
    full = np.concatenate(
        [np.asarray(res.results[c]["out"]).astype(np.float32) for c in range(NCORES)],
        axis=0)
    if post is not None:
        full = (full * post).astype(np.float32)
    return full
